# revision 1
# baseline (speedup 1.0000x reference)
"""FM model (embedding_lookup) Trainium2 Bass kernel.

Strategy: data-parallel over batch across 8 NeuronCores with a replicated,
host-augmented table.

Host side:
  - T[f*V + v, 0:64] = W_embed[f, v, :],  T[f*V + v, 64] = W_lin[f, v]
    (row stride 68 floats = 272 B, 16B-aligned rows).
  - flat indices idx[b, f] = f*V + x[b, f] (int32).
  - batch sharded 16384 -> 8 x 2048.

Device side (per core, per 128-row tile):
  - one indirect DMA gathers 128*24 rows of 68 f32 from the table into an
    SBUF tile E[128, 24*68] (partition p = batch row, fields consecutive).
  - DVE: sum_embed[p, d] = sum_f E[p, f*68+d]         (strided reduce)
         first_order[p]  = sum_f E[p, f*68+64]
  - ACT: ssqe[p] = sum_{f,d} E[p, f*68+d]^2          (Square + accum_out)
  - DVE: part[p] = bias + 0.5*||sum_embed||^2 (tensor_tensor_reduce with
         init=bias+first_order... see code)
  - ACT: out = Sigmoid(-0.5*ssqe + (first_order + bias + 0.5*sq))
"""

import sys

if "/opt/trn_rl_repo" not in sys.path:
    sys.path.insert(0, "/opt/trn_rl_repo")

import numpy as np

F = 24
V = 100000
D = 64
B = 16384
N_CORES = 8
BPC = B // N_CORES  # batch rows per core
STRIDE = 68  # f32 per augmented table row (64 emb + 1 lin + 3 pad)
P = 128

_CACHE = {}


def _build(bpc=BPC, v=V):
    import concourse.bacc as bacc
    import concourse.bass as bass
    import concourse.tile as tile
    from concourse import mybir

    ntiles = bpc // P
    nc = bacc.Bacc(
        "TRN2", target_bir_lowering=False, debug=False, num_devices=N_CORES
    )
    V_ = v
    idx = nc.dram_tensor("idx", [bpc, F], mybir.dt.int32, kind="ExternalInput").ap()
    table = nc.dram_tensor(
        "table", [F * V_, STRIDE], mybir.dt.float32, kind="ExternalInput"
    ).ap()
    biasr = nc.dram_tensor(
        "biasr", [P, 1], mybir.dt.float32, kind="ExternalInput"
    ).ap()
    out = nc.dram_tensor("out", [bpc, 1], mybir.dt.float32, kind="ExternalOutput").ap()

    fp32 = mybir.dt.float32

    with tile.TileContext(nc) as tc:
        with (
            tc.tile_pool(name="persist", bufs=1) as persist,
            tc.tile_pool(name="gather", bufs=3) as gpool,
            tc.tile_pool(name="scratch", bufs=2) as spool,
        ):
            idx_all = persist.tile([P, ntiles * F], mybir.dt.int32)
            nc.sync.dma_start(
                out=idx_all[:].rearrange("p (t f) -> p t f", t=ntiles, f=F),
                in_=idx.rearrange("(t p) f -> p t f", p=P),
            )
            bias_t = persist.tile([P, 1], fp32)
            nc.sync.dma_start(out=bias_t[:], in_=biasr[:, :])

            for t in range(ntiles):
                E = gpool.tile([P, F * STRIDE], fp32, tag="E")
                for f in range(F):
                    # HW indirect DMA: one descriptor per partition, offset
                    # taken from each partition's first offset-AP element.
                    nc.gpsimd.indirect_dma_start(
                        out=E[:, f * STRIDE : (f + 1) * STRIDE],
                        out_offset=None,
                        in_=table[:],
                        in_offset=bass.IndirectOffsetOnAxis(
                            ap=idx_all[:, t * F + f : t * F + f + 1], axis=0
                        ),
                    )
                # [p, c, f] view: element (p, c, f) = E[p, f*STRIDE + c]
                ecf = E[:].rearrange("p (f c) -> p c f", f=F, c=STRIDE)
                # [p, f, c] view for ACT square pass
                efc = E[:].rearrange("p (f c) -> p f c", f=F, c=STRIDE)

                # sum_embed[p, d] = sum_f emb  -> [P, D]
                se = spool.tile([P, D], fp32, tag="se")
                nc.vector.tensor_reduce(
                    out=se[:],
                    in_=ecf[:, 0:D, :],
                    axis=mybir.AxisListType.X,
                    op=mybir.AluOpType.add,
                )
                # first_order[p] = sum_f lin -> [P, 1]
                fo = spool.tile([P, 1], fp32, tag="fo")
                nc.vector.tensor_reduce(
                    out=fo[:],
                    in_=ecf[:, D : D + 1, :],
                    axis=mybir.AxisListType.X,
                    op=mybir.AluOpType.add,
                )
                # ssqe[p] = sum_{f,d} emb^2 (ACT: Square + accumulate)
                sq_scr = spool.tile([P, F * D], fp32, tag="sq_scr")
                ssqe = spool.tile([P, 1], fp32, tag="ssqe")
                nc.scalar.activation(
                    out=sq_scr[:].rearrange("p (f c) -> p f c", f=F, c=D),
                    in_=efc[:, :, 0:D],
                    func=mybir.ActivationFunctionType.Square,
                    accum_out=ssqe[:],
                )
                # part[p] = 0.5*||sum_embed||^2 + (first_order + bias)
                fob = spool.tile([P, 1], fp32, tag="fob")
                nc.vector.tensor_add(out=fob[:], in0=fo[:], in1=bias_t[:])
                se_sq = spool.tile([P, D], fp32, tag="se_sq")
                sq = spool.tile([P, 1], fp32, tag="sq")
                nc.scalar.activation(
                    out=se_sq[:],
                    in_=se[:],
                    func=mybir.ActivationFunctionType.Square,
                    accum_out=sq[:],
                )
                part = spool.tile([P, 1], fp32, tag="part")
                nc.vector.scalar_tensor_tensor(
                    out=part[:],
                    in0=sq[:],
                    scalar=0.5,
                    in1=fob[:],
                    op0=mybir.AluOpType.mult,
                    op1=mybir.AluOpType.add,
                )
                # out = sigmoid(-0.5*ssqe + part)
                res = spool.tile([P, 1], fp32, tag="res")
                nc.scalar.activation(
                    out=res[:],
                    in_=ssqe[:],
                    func=mybir.ActivationFunctionType.Sigmoid,
                    bias=part[:],
                    scale=-0.5,
                )
                nc.sync.dma_start(out=out[t * P : (t + 1) * P, :], in_=res[:])
    nc.compile()
    return nc


def _get_nc(bpc=BPC):
    if bpc not in _CACHE:
        _CACHE[bpc] = _build(bpc)
    return _CACHE[bpc]


def _prep_inputs(x, W_embed, W_lin, bias):
    x = np.asarray(x)
    W_embed = np.asarray(W_embed, dtype=np.float32)
    W_lin = np.asarray(W_lin, dtype=np.float32)
    bias = np.asarray(bias, dtype=np.float32)
    assert x.shape == (B, F), x.shape

    tab = np.empty((F * V, STRIDE), dtype=np.float32)
    tab[:, :D] = W_embed.reshape(F * V, D)
    tab[:, D] = W_lin.reshape(F * V)
    tab[:, D + 1 :] = 0.0

    flat = (x.astype(np.int64) + (np.arange(F, dtype=np.int64) * V)[None, :]).astype(
        np.int32
    )
    bias_rep = np.full((P, 1), float(bias.reshape(-1)[0]), dtype=np.float32)

    in_maps = [
        {
            "idx": np.ascontiguousarray(flat[i * BPC : (i + 1) * BPC]),
            "table": tab,
            "biasr": bias_rep,
        }
        for i in range(N_CORES)
    ]
    return in_maps


def _run(in_maps, trace=False, tmpdir=None):
    from concourse.bass_utils import run_bass_kernel_spmd

    nc = _get_nc()
    res = run_bass_kernel_spmd(
        nc, in_maps, list(range(N_CORES)), trace=trace, tmpdir=tmpdir
    )
    outs = [res.results[i]["out"] for i in range(N_CORES)]
    return np.concatenate(outs, axis=0), res


def kernel(x, W_embed, W_lin, bias):
    in_maps = _prep_inputs(x, W_embed, W_lin, bias)
    out, _ = _run(in_maps)
    return out



# revision 4
# speedup vs baseline: 1.0019x; 1.0019x over previous
"""FM model (embedding_lookup) Trainium2 Bass kernel.

Strategy: data-parallel over batch across 8 NeuronCores with a replicated,
host-augmented table.

Host side:
  - T[f*V + v] = [W_embed[f,v,:] (64) | W_lin[f,v] | sum_d W_embed[f,v,d]^2
    | pad | pad]  (row stride 68 f32 = 272 B, 16B-aligned rows).
  - flat indices idx[b, f] = f*V + x[b, f] (int32).
  - batch sharded 16384 -> 8 x 2048.

Device side (per core, per 128-row tile of batch):
  - 24 HW indirect DMAs (one per field, 128 descriptors each: one offset
    per partition) gather the augmented rows into E[128, 24*68].
    This is SWDGE-Q7-bound: descriptor generation for data-dependent
    addresses runs at ~7.5 ns/descriptor + ~310 ns dispatch per
    instruction, which sets the kernel floor (~49k descriptors/core).
  - DVE: s[p, c] = sum_f E[p, f*68+c] -> s = [sum_embed(64) | first_order
    | ssqe | junk]  (single strided reduce).
  - ACT: sq[p] = ||s[p,0:64]||^2 (Square + accum_out)
  - DVE: pre[p] = -0.5*s[p,65] + s[p,64]; preb = pre + bias
  - ACT: out = Sigmoid(0.5*sq + preb)
"""

import sys

if "/opt/trn_rl_repo" not in sys.path:
    sys.path.insert(0, "/opt/trn_rl_repo")

import numpy as np

F = 24
V = 100000
D = 64
B = 16384
N_CORES = 8
BPC = B // N_CORES  # batch rows per core
STRIDE = 68  # f32 per augmented table row (64 emb + 1 lin + 1 rowsq + 2 pad)
P = 128

_CACHE = {}


def _build(bpc=BPC, v=V):
    import concourse.bacc as bacc
    import concourse.bass as bass
    import concourse.tile as tile
    from concourse import mybir

    ntiles = bpc // P
    nc = bacc.Bacc(
        "TRN2", target_bir_lowering=False, debug=False, num_devices=N_CORES
    )
    V_ = v
    idx = nc.dram_tensor("idx", [bpc, F], mybir.dt.int32, kind="ExternalInput").ap()
    table = nc.dram_tensor(
        "table", [F * V_, STRIDE], mybir.dt.float32, kind="ExternalInput"
    ).ap()
    biasr = nc.dram_tensor(
        "biasr", [P, 1], mybir.dt.float32, kind="ExternalInput"
    ).ap()
    out = nc.dram_tensor("out", [bpc, 1], mybir.dt.float32, kind="ExternalOutput").ap()

    fp32 = mybir.dt.float32

    with tile.TileContext(nc) as tc:
        with (
            tc.tile_pool(name="persist", bufs=1) as persist,
            tc.tile_pool(name="gather", bufs=4) as gpool,
            tc.tile_pool(name="scratch", bufs=3) as spool,
        ):
            idx_all = persist.tile([P, ntiles * F], mybir.dt.int32)
            nc.sync.dma_start(
                out=idx_all[:].rearrange("p (t f) -> p t f", t=ntiles, f=F),
                in_=idx.rearrange("(t p) f -> p t f", p=P),
            )
            bias_t = persist.tile([P, 1], fp32)
            nc.sync.dma_start(out=bias_t[:], in_=biasr[:, :])

            res_all = persist.tile([P, ntiles], fp32)

            for t in range(ntiles):
                E = gpool.tile([P, F * STRIDE], fp32, tag="E")
                for f in range(F):
                    nc.gpsimd.indirect_dma_start(
                        out=E[:, f * STRIDE : (f + 1) * STRIDE],
                        out_offset=None,
                        in_=table[:],
                        in_offset=bass.IndirectOffsetOnAxis(
                            ap=idx_all[:, t * F + f : t * F + f + 1], axis=0
                        ),
                    )
                # s[p, c] = sum_f E[p, f*68+c]
                ecf = E[:].rearrange("p (f c) -> p c f", f=F, c=STRIDE)
                s = spool.tile([P, STRIDE], fp32, tag="s")
                nc.vector.tensor_reduce(
                    out=s[:],
                    in_=ecf[:, :, :],
                    axis=mybir.AxisListType.X,
                    op=mybir.AluOpType.add,
                )
                # sq[p] = ||sum_embed||^2
                se_sq = spool.tile([P, D], fp32, tag="se_sq")
                sq = spool.tile([P, 1], fp32, tag="sq")
                nc.scalar.activation(
                    out=se_sq[:],
                    in_=s[:, 0:D],
                    func=mybir.ActivationFunctionType.Square,
                    accum_out=sq[:],
                )
                # pre = -0.5*ssqe + first_order;  preb = pre + bias
                pre = spool.tile([P, 1], fp32, tag="pre")
                nc.vector.scalar_tensor_tensor(
                    out=pre[:],
                    in0=s[:, D + 1 : D + 2],
                    scalar=-0.5,
                    in1=s[:, D : D + 1],
                    op0=mybir.AluOpType.mult,
                    op1=mybir.AluOpType.add,
                )
                preb = spool.tile([P, 1], fp32, tag="preb")
                nc.vector.tensor_add(out=preb[:], in0=pre[:], in1=bias_t[:])
                # out = sigmoid(0.5*sq + preb)
                nc.scalar.activation(
                    out=res_all[:, t : t + 1],
                    in_=sq[:],
                    func=mybir.ActivationFunctionType.Sigmoid,
                    bias=preb[:],
                    scale=0.5,
                )
            nc.sync.dma_start(
                out=out.rearrange("(t p) o -> p (t o)", p=P),
                in_=res_all[:],
            )
    nc.compile()
    return nc


def _get_nc(bpc=BPC):
    if bpc not in _CACHE:
        _CACHE[bpc] = _build(bpc)
    return _CACHE[bpc]


def _prep_inputs(x, W_embed, W_lin, bias):
    x = np.asarray(x)
    W_embed = np.asarray(W_embed, dtype=np.float32)
    W_lin = np.asarray(W_lin, dtype=np.float32)
    bias = np.asarray(bias, dtype=np.float32)
    assert x.shape == (B, F), x.shape

    We = W_embed.reshape(F * V, D)
    tab = np.empty((F * V, STRIDE), dtype=np.float32)
    tab[:, :D] = We
    tab[:, D] = W_lin.reshape(F * V)
    tab[:, D + 1] = np.einsum("rd,rd->r", We, We)
    tab[:, D + 2 :] = 0.0

    flat = (x.astype(np.int64) + (np.arange(F, dtype=np.int64) * V)[None, :]).astype(
        np.int32
    )
    bias_rep = np.full((P, 1), float(bias.reshape(-1)[0]), dtype=np.float32)

    in_maps = [
        {
            "idx": np.ascontiguousarray(flat[i * BPC : (i + 1) * BPC]),
            "table": tab,
            "biasr": bias_rep,
        }
        for i in range(N_CORES)
    ]
    return in_maps


def _run(in_maps, trace=False, tmpdir=None):
    from concourse.bass_utils import run_bass_kernel_spmd

    nc = _get_nc()
    res = run_bass_kernel_spmd(
        nc, in_maps, list(range(N_CORES)), trace=trace, tmpdir=tmpdir
    )
    outs = [res.results[i]["out"] for i in range(N_CORES)]
    return np.concatenate(outs, axis=0), res


def kernel(x, W_embed, W_lin, bias):
    in_maps = _prep_inputs(x, W_embed, W_lin, bias)
    out, _ = _run(in_maps)
    return out
